# revision 6
# baseline (speedup 1.0000x reference)
"""Causal self-attention with RoPE on 8 Trainium2 NeuronCores.

Reference computation (B=4, T=2048, D=1024, H=16, hd=64, fp32):
    qkv = x @ w_qkv ; q,k per-head RoPE (interleaved pairs) ;
    out = softmax(causal(q k^T / 8)) @ v ; out @ w_proj

Sharding: core c -> (batch b = c//2, head-group g = c%2 of 8 heads).
Data parallel on B, tensor parallel on heads; w_proj is row-parallel so each
core returns a partial [2048, 1024] product and the host sums the two
partials per batch (the "all-reduce" of the row-parallel linear).

Per-core device program (all matmuls in fp32r: fp32 with 11-bit mantissa
round, full PE speed; accumulation fp32 in PSUM):
  1. x^T via PE transposes (x pre-rounded to fp32r on host).
  2. qkv: q,k in transposed layout [feat, t] (lhsT = w tiles, rhs = x^T);
     v in natural layout [t, feat] (lhsT = x^T tiles, rhs = w_v).
     The q/k weight matrix is augmented host-side with rotated duplicates
     (q' = [-w_odd | w_even] per head) so RoPE becomes
       q_rot = q * cc + q' * ss      (2 DVE mults + 1 add, no partition swap)
     with cc/ss = cos/sin tables replicated across the 4x32 partition rows.
     The 1/sqrt(hd) score scale is folded into the k columns host-side.
  3. Attention per head in transposed-score layout: S^T[j, i] tiles
     (j = keys on partitions), causal mask as additive -3e38 on the diagonal
     128x128 block, exp on ScalarE straight into fp32r P^T tiles, then
     out^T[d, i] accumulated as matmul(lhsT=[v | ones], rhs=P^T) -- the ones
     column yields the softmax denominators in row 64 for free.
     Normalize: reciprocal_approx_fast + gpsimd partition_broadcast + mult.
  4. out_partial = attnT @ w_proj (row-parallel slice), DMA to DRAM.
"""

import numpy as np

import concourse.bass as bass
import concourse.tile as tile
from concourse import bacc, mybir
from concourse.bass_utils import run_bass_kernel_spmd
from neuron_dtypes._impl import fp32r as fp32r_impl

F32 = mybir.dt.float32
F32R = mybir.dt.float32r
AF = mybir.ActivationFunctionType
OP = mybir.AluOpType

B, T, D, NH, HD = 4, 2048, 1024, 16, 64
HPC = 8            # heads per core
NEG = -3.0e38
N_CORES = 8
NTT = T // 128     # 16 token tiles
NKT = D // 128     # 8 contraction tiles


def _round_fp32r(x: np.ndarray) -> np.ndarray:
    xb = np.ascontiguousarray(x).view(np.uint32).ravel()
    r = np.asarray(fp32r_impl.cast_fp32_to_fp32r(len(xb), xb), dtype=np.uint32)
    return r.view(np.float32).reshape(x.shape)


def _build_program():
    nc = bacc.Bacc("TRN2", target_bir_lowering=False, debug=False)
    x_d = nc.dram_tensor("x", [T, D], F32R, kind="ExternalInput")
    wqk_d = nc.dram_tensor("wqk", [D, 2048], F32R, kind="ExternalInput")
    wv_d = nc.dram_tensor("wv", [D, 512], F32R, kind="ExternalInput")
    wp_d = nc.dram_tensor("wproj", [512, D], F32R, kind="ExternalInput")
    cc_d = nc.dram_tensor("cc", [128, T], F32, kind="ExternalInput")
    ss_d = nc.dram_tensor("ss", [128, T], F32, kind="ExternalInput")
    tri_d = nc.dram_tensor("tri", [128, 128], F32, kind="ExternalInput")
    id_d = nc.dram_tensor("ident", [128, 128], F32R, kind="ExternalInput")
    out_d = nc.dram_tensor("out", [T, D], F32, kind="ExternalOutput")

    with tile.TileContext(nc) as tc:
        with (
            tc.tile_pool(name="persist", bufs=1) as pers,
            tc.tile_pool(name="vo", bufs=1) as vop,
            tc.tile_pool(name="qkt", bufs=1) as qktp,
        ):
            ident = pers.tile([128, 128], F32R, tag="ident")
            tri = pers.tile([128, 128], F32, tag="tri")
            nc.sync.dma_start(ident[:], id_d[:])
            nc.sync.dma_start(tri[:], tri_d[:])

            # [128, h, 65] per token tile: v columns 0:64, ones at col 64
            vo = [vop.tile([128, HPC, 65], F32R, tag=f"vo{tt}", name=f"vo{tt}") for tt in range(NTT)]

            with (
                tc.tile_pool(name="xt", bufs=1) as xtp,
            ):
                # ---- phase A: x -> x^T --------------------------------------
                xt = [xtp.tile([128, T], F32R, tag=f"xt{kt}", name=f"xt{kt}") for kt in range(NKT)]
                with (
                    tc.tile_pool(name="xnat", bufs=1) as xnp,
                    tc.tile_pool(name="xtps", bufs=4, space="PSUM") as xtps,
                ):
                    for tg in range(NTT // 4):
                        xn = []
                        for tl in range(4):
                            t_ = xnp.tile([128, D], F32R, tag=f"xn{tl}", name=f"xn{tg}_{tl}")
                            nc.sync.dma_start(
                                t_[:], x_d[(tg * 4 + tl) * 128:(tg * 4 + tl + 1) * 128, :]
                            )
                            xn.append(t_)
                        for kt in range(NKT):
                            ps = xtps.tile([128, 512], F32R, tag="xtps")
                            for tl in range(4):
                                nc.tensor.transpose(
                                    ps[:, tl * 128:(tl + 1) * 128],
                                    xn[tl][:, kt * 128:(kt + 1) * 128],
                                    ident[:],
                                )
                            nc.scalar.copy(
                                xt[kt][:, tg * 512:(tg + 1) * 512], ps[:]
                            )

                # ---- phase B: qkv + rope ------------------------------------
                with (
                    tc.tile_pool(name="wvp", bufs=1) as wvp,
                    tc.tile_pool(name="vps", bufs=2, space="PSUM") as vps,
                ):
                    wv_sb = wvp.tile([128, NKT, 512], F32R, tag="wv")
                    nc.sync.dma_start(
                        wv_sb[:], wv_d.rearrange("(t p) f -> p t f", p=128)
                    )

                    # v (natural layout) + ones column
                    for tt in range(NTT):
                        nc.vector.memset(vo[tt][:].bitcast(F32), 1.0)
                        ps = vps.tile([128, 512], F32, tag="vps")
                        for kt in range(NKT):
                            nc.tensor.matmul(
                                ps[:],
                                xt[kt][:, tt * 128:(tt + 1) * 128],
                                wv_sb[:, kt, :],
                                start=(kt == 0), stop=(kt == NKT - 1),
                            )
                        nc.vector.tensor_copy(
                            vo[tt][:, :, 0:64],
                            ps[:].rearrange("p (h d) -> p h d", h=HPC),
                        )

                with (
                    tc.tile_pool(name="ccss", bufs=1) as ccssp,
                    tc.tile_pool(name="ftw", bufs=2) as ftwp,
                    tc.tile_pool(name="ropet", bufs=2) as ropetp,
                    tc.tile_pool(name="qkps", bufs=4, space="PSUM") as qkps,
                ):
                    cc = ccssp.tile([128, T], F32, tag="cc")
                    ss = ccssp.tile([128, T], F32, tag="ss")
                    nc.sync.dma_start(cc[:], cc_d[:])
                    nc.sync.dma_start(ss[:], ss_d[:])

                    # q/k transposed + rope
                    # qkt tiles 0..3 = roped q pairs, 4..7 = roped k pairs
                    qkt = [
                        qktp.tile([128, T], F32R, tag=f"qkt{i}", name=f"qkt{i}")
                        for i in range(8)
                    ]
                    for side in range(2):          # 0 = q, 1 = k
                        for pr in range(4):        # head pair
                            ft = 8 * side + pr
                            ftr = ft + 4           # rotated duplicate block
                            w_a = ftwp.tile([128, NKT, 128], F32R, tag="wa")
                            w_b = ftwp.tile([128, NKT, 128], F32R, tag="wb")
                            wr = wqk_d.rearrange("(t p) f -> p t f", p=128)
                            nc.sync.dma_start(w_a[:], wr[:, :, ft * 128:(ft + 1) * 128])
                            nc.sync.dma_start(w_b[:], wr[:, :, ftr * 128:(ftr + 1) * 128])
                            for tcn in range(4):
                                sl = slice(tcn * 512, (tcn + 1) * 512)
                                ps_a = qkps.tile([128, 512], F32, tag="qkps")
                                ps_b = qkps.tile([128, 512], F32, tag="qkps")
                                for kt in range(NKT):
                                    nc.tensor.matmul(
                                        ps_a[:], w_a[:, kt, :], xt[kt][:, sl],
                                        start=(kt == 0), stop=(kt == NKT - 1),
                                    )
                                for kt in range(NKT):
                                    nc.tensor.matmul(
                                        ps_b[:], w_b[:, kt, :], xt[kt][:, sl],
                                        start=(kt == 0), stop=(kt == NKT - 1),
                                    )
                                t1 = ropetp.tile([128, 512], F32, tag="t1")
                                t2 = ropetp.tile([128, 512], F32, tag="t2")
                                nc.vector.tensor_tensor(t1[:], ps_a[:], cc[:, sl], OP.mult)
                                nc.vector.tensor_tensor(t2[:], ps_b[:], ss[:, sl], OP.mult)
                                nc.vector.tensor_tensor(
                                    qkt[4 * side + pr][:, sl], t1[:], t2[:], OP.add
                                )

            # ---- phase C: attention + projection ----------------------------
            with (
                tc.tile_pool(name="wpp", bufs=1) as wpp,
                tc.tile_pool(name="atn", bufs=2) as atnp,
                tc.tile_pool(name="pt", bufs=3) as ptp,
                tc.tile_pool(name="nrm", bufs=2) as nrmp,
                tc.tile_pool(name="osb", bufs=2) as osbp,
                tc.tile_pool(name="stps", bufs=2, space="PSUM") as stps,
                tc.tile_pool(name="atps", bufs=2, space="PSUM") as atps,
            ):
                wp_sb = wpp.tile([128, 4, D], F32R, tag="wp")
                nc.sync.dma_start(wp_sb[:], wp_d.rearrange("(t p) f -> p t f", p=128))

                for ih in range(2):
                    i0 = 1024 * ih
                    at_tiles = [
                        atnp.tile([128, 1024], F32R, tag=f"at{pr}", name=f"at{ih}_{pr}")
                        for pr in range(4)
                    ]
                    for h in range(HPC):
                        pr, r0 = h // 2, 64 * (h % 2)
                        qt_ap = qkt[pr][r0:r0 + 64, :]
                        kt_ap = qkt[4 + pr][r0:r0 + 64, :]
                        at_ps = atps.tile([65, 1024], F32, tag="atps")
                        n_jt = 8 * ih + 8
                        for jt in range(n_jt):
                            j0 = 128 * jt
                            i_lo = max(i0, j0)
                            segs = []
                            lo = i_lo
                            while lo < i0 + 1024:
                                hi = min(i0 + 1024, (lo // 512 + 1) * 512)
                                segs.append((lo, hi))
                                lo = hi
                            st = stps.tile([128, 1024], F32, tag="st")
                            for (lo, hi) in segs:
                                nc.tensor.matmul(
                                    st[:, lo - i0:hi - i0],
                                    kt_ap[:, j0:j0 + 128],
                                    qt_ap[:, lo:hi],
                                    start=True, stop=True,
                                )
                            if j0 >= i0:
                                nc.vector.tensor_tensor(
                                    st[:, j0 - i0:j0 - i0 + 128],
                                    st[:, j0 - i0:j0 - i0 + 128],
                                    tri[:], OP.add,
                                )
                            pt = ptp.tile([128, 1024], F32R, tag="pt")
                            nc.scalar.activation(
                                pt[:, 0:i0 + 1024 - i_lo], st[:, i_lo - i0:], AF.Exp
                            )
                            for (lo, hi) in segs:
                                # last jt that writes this psum bank
                                last_jt = min(n_jt - 1, (hi - 1) // 128)
                                nc.tensor.matmul(
                                    at_ps[:, lo - i0:hi - i0],
                                    vo[jt][:, h, :],
                                    pt[:, lo - i_lo:hi - i_lo],
                                    start=(jt == 0), stop=(jt == last_jt),
                                )
                        # normalize
                        sum_sb = nrmp.tile([1, 1024], F32, tag="sum")
                        r_sb = nrmp.tile([1, 1024], F32, tag="r")
                        rb_sb = nrmp.tile([64, 1024], F32, tag="rb")
                        nc.scalar.copy(sum_sb[:], at_ps[64:65, :])
                        nc.vector.reciprocal_approx_fast(r_sb[:], sum_sb[:])
                        nc.gpsimd.partition_broadcast(rb_sb[:], r_sb[:])
                        nc.vector.tensor_tensor(
                            at_tiles[pr][r0:r0 + 64, :], at_ps[0:64, :],
                            rb_sb[:], OP.mult,
                        )
                    # projection for this i-half
                    for tl in range(8):
                        tt = 8 * ih + tl
                        pp = stps.tile([128, 1024], F32, tag="st")
                        for nch in range(2):
                            for mt in range(4):
                                nc.tensor.matmul(
                                    pp[:, nch * 512:(nch + 1) * 512],
                                    at_tiles[mt][:, tl * 128:(tl + 1) * 128],
                                    wp_sb[:, mt, nch * 512:(nch + 1) * 512],
                                    start=(mt == 0), stop=(mt == 3),
                                )
                        o_sb = osbp.tile([128, 1024], F32, tag="osb")
                        nc.scalar.copy(o_sb[:], pp[:])
                        nc.sync.dma_start(out_d[tt * 128:(tt + 1) * 128, :], o_sb[:])
    nc.compile()
    return nc


_NC_CACHE = None


def _get_program():
    global _NC_CACHE
    if _NC_CACHE is None:
        _NC_CACHE = _build_program()
    return _NC_CACHE


def _host_inputs(x, cos, sin, w_qkv, w_proj):
    """Build the 8 per-core input dicts."""
    x = np.asarray(x, np.float32)
    cos = np.asarray(cos, np.float32)
    sin = np.asarray(sin, np.float32)
    w_qkv = np.asarray(w_qkv, np.float32)
    w_proj = np.asarray(w_proj, np.float32)

    cct = np.tile(cos.T, (4, 1)).astype(np.float32)          # [128, T]
    sst = np.tile(sin.T, (4, 1)).astype(np.float32)
    tri = np.where(
        np.arange(128)[None, :] >= np.arange(128)[:, None], 0.0, NEG
    ).astype(np.float32)
    ident = np.eye(128, dtype=np.float32)

    x_r = [_round_fp32r(np.ascontiguousarray(x[b])) for b in range(B)]

    wq = w_qkv[:, 0:D]
    wk = w_qkv[:, D:2 * D] * np.float32(1.0 / np.sqrt(HD))
    wv = w_qkv[:, 2 * D:3 * D]

    def build_qk_aug(g):
        cols = []
        # ft 0..3: q pairs (evens then odds per head)
        for blk, w in ((0, wq), (1, wk)):
            plain, rot = [], []
            for pr in range(4):
                p_cols, r_cols = [], []
                for hl in (0, 1):
                    hw = w[:, (g * 8 + 2 * pr + hl) * 64:(g * 8 + 2 * pr + hl + 1) * 64]
                    ev, od = hw[:, 0::2], hw[:, 1::2]
                    p_cols.append(np.concatenate([ev, od], axis=1))
                    r_cols.append(np.concatenate([-od, ev], axis=1))
                plain.append(np.concatenate(p_cols, axis=1))
                rot.append(np.concatenate(r_cols, axis=1))
            cols.extend(plain)
            cols.extend(rot)
        return np.concatenate(cols, axis=1)  # [D, 2048]

    wqk_g = [_round_fp32r(build_qk_aug(g)) for g in range(2)]
    wv_g = [_round_fp32r(np.ascontiguousarray(wv[:, g * 512:(g + 1) * 512]))
            for g in range(2)]
    wp_g = [_round_fp32r(np.ascontiguousarray(w_proj[g * 512:(g + 1) * 512, :]))
            for g in range(2)]

    in_maps = []
    for c in range(N_CORES):
        b, g = c // 2, c % 2
        in_maps.append({
            "x": x_r[b], "wqk": wqk_g[g], "wv": wv_g[g], "wproj": wp_g[g],
            "cc": cct, "ss": sst, "tri": tri, "ident": _round_fp32r(ident),
        })
    return in_maps


def kernel(x, cos, sin, w_qkv, w_proj):
    nc = _get_program()
    in_maps = _host_inputs(x, cos, sin, w_qkv, w_proj)
    res = run_bass_kernel_spmd(nc, in_maps, core_ids=list(range(N_CORES)))
    out = np.empty((B, T, D), dtype=np.float32)
    for b in range(B):
        out[b] = res.results[2 * b]["out"] + res.results[2 * b + 1]["out"]
    return out


# revision 8
# speedup vs baseline: 1.1219x; 1.1219x over previous
"""Causal self-attention with RoPE on 8 Trainium2 NeuronCores.

Reference computation (B=4, T=2048, D=1024, H=16, hd=64, fp32):
    qkv = x @ w_qkv ; q,k per-head RoPE (interleaved pairs) ;
    out = softmax(causal(q k^T / 8)) @ v ; out @ w_proj

Sharding: core c -> (batch b = c//2, head-group g = c%2 of 8 heads).
Data parallel on B, tensor parallel on heads; w_proj is row-parallel so each
core returns a partial [2048, 1024] product and the host sums the two
partials per batch (the "all-reduce" of the row-parallel linear).

Per-core device program (all matmuls in fp32r: fp32 with 11-bit mantissa
round, full PE speed; accumulation fp32 in PSUM):
  1. x^T via PE transposes (x pre-rounded to fp32r on host).
  2. qkv: q,k in transposed layout [feat, t] (lhsT = w tiles, rhs = x^T);
     v in natural layout [t, feat] (lhsT = x^T tiles, rhs = w_v).
     The q/k weight matrix is augmented host-side with rotated duplicates
     (q' = [-w_odd | w_even] per head) so RoPE becomes
       q_rot = q * cc + q' * ss      (2 DVE mults + 1 add, no partition swap)
     with cc/ss = cos/sin tables replicated across the 4x32 partition rows.
     The 1/sqrt(hd) score scale is folded into the k columns host-side.
  3. Attention per head in transposed-score layout: S^T[j, i] tiles
     (j = keys on partitions), causal mask as additive -3e38 on the diagonal
     128x128 block, exp on ScalarE straight into fp32r P^T tiles, then
     out^T[d, i] accumulated as matmul(lhsT=[v | ones], rhs=P^T) -- the ones
     column yields the softmax denominators in row 64 for free.
     Normalize: reciprocal_approx_fast + gpsimd partition_broadcast + mult.
  4. out_partial = attnT @ w_proj (row-parallel slice), DMA to DRAM.
"""

import numpy as np

import concourse.bass as bass
import concourse.tile as tile
from concourse import bacc, mybir
from concourse.bass_utils import run_bass_kernel_spmd
from neuron_dtypes._impl import fp32r as fp32r_impl

F32 = mybir.dt.float32
F32R = mybir.dt.float32r
AF = mybir.ActivationFunctionType
OP = mybir.AluOpType

B, T, D, NH, HD = 4, 2048, 1024, 16, 64
HPC = 8            # heads per core
NEG = -3.0e38
N_CORES = 8
NTT = T // 128     # 16 token tiles
NKT = D // 128     # 8 contraction tiles


def _round_fp32r(x: np.ndarray) -> np.ndarray:
    xb = np.ascontiguousarray(x).view(np.uint32).ravel()
    r = np.asarray(fp32r_impl.cast_fp32_to_fp32r(len(xb), xb), dtype=np.uint32)
    return r.view(np.float32).reshape(x.shape)


def _build_program(reps: int = 1):
    nc = bacc.Bacc("TRN2", target_bir_lowering=False, debug=False)
    x_d = nc.dram_tensor("x", [T, D], F32R, kind="ExternalInput")
    wqk_d = nc.dram_tensor("wqk", [D, 1024], F32R, kind="ExternalInput")
    perm_d = nc.dram_tensor("perm", [128, 128], F32R, kind="ExternalInput")
    wv_d = nc.dram_tensor("wv", [D, 512], F32R, kind="ExternalInput")
    wp_d = nc.dram_tensor("wproj", [512, D], F32R, kind="ExternalInput")
    cc_d = nc.dram_tensor("cc", [128, T], F32, kind="ExternalInput")
    ss_d = nc.dram_tensor("ss", [128, T], F32, kind="ExternalInput")
    tri_d = nc.dram_tensor("tri", [128, 128], F32, kind="ExternalInput")
    id_d = nc.dram_tensor("ident", [128, 128], F32R, kind="ExternalInput")
    out_d = nc.dram_tensor("out", [T, D], F32, kind="ExternalOutput")

    with tile.TileContext(nc) as tc:
      for _rep in range(reps):
        with (
            tc.tile_pool(name="persist", bufs=1) as pers,
            tc.tile_pool(name="vo", bufs=1) as vop,
            tc.tile_pool(name="qkt", bufs=1) as qktp,
        ):
            ident = pers.tile([128, 128], F32R, tag="ident")
            tri = pers.tile([128, 128], F32, tag="tri")
            perm = pers.tile([128, 128], F32R, tag="perm")
            nc.sync.dma_start(ident[:], id_d[:])
            nc.sync.dma_start(tri[:], tri_d[:])
            nc.sync.dma_start(perm[:], perm_d[:])

            # [128, h, 65] per token tile: v columns 0:64, ones at col 64
            vo = [vop.tile([128, HPC, 65], F32R, tag=f"vo{tt}", name=f"vo{tt}") for tt in range(NTT)]

            with (
                tc.tile_pool(name="xt", bufs=1) as xtp,
            ):
                # ---- phase A: x -> x^T --------------------------------------
                xt = [xtp.tile([128, T], F32R, tag=f"xt{kt}", name=f"xt{kt}") for kt in range(NKT)]
                with (
                    tc.tile_pool(name="xnat", bufs=1) as xnp,
                    tc.tile_pool(name="xtps", bufs=4, space="PSUM") as xtps,
                ):
                    for tg in range(NTT // 4):
                        xn = []
                        for tl in range(4):
                            t_ = xnp.tile([128, D], F32R, tag=f"xn{tl}", name=f"xn{tg}_{tl}")
                            nc.sync.dma_start(
                                t_[:], x_d[(tg * 4 + tl) * 128:(tg * 4 + tl + 1) * 128, :]
                            )
                            xn.append(t_)
                        for kt in range(NKT):
                            ps = xtps.tile([128, 512], F32R, tag="xtps")
                            for tl in range(4):
                                nc.tensor.transpose(
                                    ps[:, tl * 128:(tl + 1) * 128],
                                    xn[tl][:, kt * 128:(kt + 1) * 128],
                                    ident[:],
                                )
                            nc.vector.tensor_copy(
                                xt[kt][:, tg * 512:(tg + 1) * 512], ps[:]
                            )

                # ---- phase B: qkv + rope ------------------------------------
                with (
                    tc.tile_pool(name="wvp", bufs=1) as wvp,
                    tc.tile_pool(name="vps", bufs=2, space="PSUM") as vps,
                ):
                    wv_sb = wvp.tile([128, NKT, 512], F32R, tag="wv")
                    nc.sync.dma_start(
                        wv_sb[:], wv_d.rearrange("(t p) f -> p t f", p=128)
                    )

                    # v (natural layout) + ones column
                    for tt in range(NTT):
                        nc.vector.memset(vo[tt][:].bitcast(F32), 1.0)
                        ps = vps.tile([128, 512], F32, tag="vps")
                        for kt in range(NKT):
                            nc.tensor.matmul(
                                ps[:],
                                xt[kt][:, tt * 128:(tt + 1) * 128],
                                wv_sb[:, kt, :],
                                start=(kt == 0), stop=(kt == NKT - 1),
                            )
                        nc.vector.tensor_copy(
                            vo[tt][:, :, 0:64],
                            ps[:].rearrange("p (h d) -> p h d", h=HPC),
                        )

                with (
                    tc.tile_pool(name="ccss", bufs=1) as ccssp,
                    tc.tile_pool(name="ftw", bufs=2) as ftwp,
                    tc.tile_pool(name="ropet", bufs=2) as ropetp,
                    tc.tile_pool(name="qkps", bufs=4, space="PSUM") as qkps,
                ):
                    cc = ccssp.tile([128, T], F32, tag="cc")
                    ss = ccssp.tile([128, T], F32, tag="ss")
                    nc.sync.dma_start(cc[:], cc_d[:])
                    nc.sync.dma_start(ss[:], ss_d[:])

                    # q/k transposed + rope
                    # qkt tiles 0..3 = roped q pairs, 4..7 = roped k pairs
                    qkt = [
                        qktp.tile([128, T], F32R, tag=f"qkt{i}", name=f"qkt{i}")
                        for i in range(8)
                    ]
                    for side in range(2):          # 0 = q, 1 = k
                        for pr in range(4):        # head pair
                            ft = 4 * side + pr
                            w_a = ftwp.tile([128, NKT, 128], F32R, tag="wa")
                            wr = wqk_d.rearrange("(t p) f -> p t f", p=128)
                            nc.sync.dma_start(w_a[:], wr[:, :, ft * 128:(ft + 1) * 128])
                            for tcn in range(4):
                                sl = slice(tcn * 512, (tcn + 1) * 512)
                                ps_a = qkps.tile([128, 512], F32, tag="qkps")
                                ps_b = qkps.tile([128, 512], F32, tag="qkps")
                                for kt in range(NKT):
                                    nc.tensor.matmul(
                                        ps_a[:], w_a[:, kt, :], xt[kt][:, sl],
                                        start=(kt == 0), stop=(kt == NKT - 1),
                                    )
                                q_tmp = ropetp.tile([128, 512], F32R, tag="qtmp")
                                nc.vector.tensor_copy(q_tmp[:], ps_a[:])
                                # rotated copy: ps_b = signed 32-row-block swap of q
                                nc.tensor.matmul(
                                    ps_b[:], perm[:], q_tmp[:],
                                    start=True, stop=True,
                                )
                                t1 = ropetp.tile([128, 512], F32, tag="t1")
                                t2 = ropetp.tile([128, 512], F32, tag="t2")
                                nc.vector.tensor_tensor(t1[:], ps_a[:], cc[:, sl], OP.mult)
                                nc.vector.tensor_tensor(t2[:], ps_b[:], ss[:, sl], OP.mult)
                                nc.gpsimd.tensor_tensor(
                                    qkt[4 * side + pr][:, sl], t1[:], t2[:], OP.add
                                )

            # ---- phase C: attention + projection ----------------------------
            with (
                tc.tile_pool(name="wpp", bufs=1) as wpp,
                tc.tile_pool(name="atn", bufs=2) as atnp,
                tc.tile_pool(name="pt", bufs=3) as ptp,
                tc.tile_pool(name="nrm", bufs=2) as nrmp,
                tc.tile_pool(name="osb", bufs=2) as osbp,
                tc.tile_pool(name="stps", bufs=2, space="PSUM") as stps,
                tc.tile_pool(name="atps", bufs=2, space="PSUM") as atps,
            ):
                wp_sb = wpp.tile([128, 4, D], F32R, tag="wp")
                nc.sync.dma_start(wp_sb[:], wp_d.rearrange("(t p) f -> p t f", p=128))

                for ih in range(2):
                    i0 = 1024 * ih
                    at_tiles = [
                        atnp.tile([128, 1024], F32R, tag=f"at{pr}", name=f"at{ih}_{pr}")
                        for pr in range(4)
                    ]
                    for h in range(HPC):
                        pr, r0 = h // 2, 64 * (h % 2)
                        qt_ap = qkt[pr][r0:r0 + 64, :]
                        kt_ap = qkt[4 + pr][r0:r0 + 64, :]
                        at_ps = atps.tile([65, 1024], F32, tag="atps")
                        n_jt = 8 * ih + 8
                        for jt in range(n_jt):
                            j0 = 128 * jt
                            i_lo = max(i0, j0)
                            segs = []
                            lo = i_lo
                            while lo < i0 + 1024:
                                hi = min(i0 + 1024, (lo // 512 + 1) * 512)
                                segs.append((lo, hi))
                                lo = hi
                            st = stps.tile([128, 1024], F32, tag="st")
                            for (lo, hi) in segs:
                                nc.tensor.matmul(
                                    st[:, lo - i0:hi - i0],
                                    kt_ap[:, j0:j0 + 128],
                                    qt_ap[:, lo:hi],
                                    start=True, stop=True,
                                )
                            pt = ptp.tile([128, 1024], F32R, tag="pt")
                            nc.scalar.activation(
                                pt[:, 0:i0 + 1024 - i_lo], st[:, i_lo - i0:], AF.Exp
                            )
                            if j0 >= i0:
                                nc.gpsimd.tensor_tensor(
                                    pt[:, 0:128], pt[:, 0:128], tri[:], OP.mult,
                                )
                            for (lo, hi) in segs:
                                # last jt that writes this psum bank
                                last_jt = min(n_jt - 1, (hi - 1) // 128)
                                nc.tensor.matmul(
                                    at_ps[:, lo - i0:hi - i0],
                                    vo[jt][:, h, :],
                                    pt[:, lo - i_lo:hi - i_lo],
                                    start=(jt == 0), stop=(jt == last_jt),
                                )
                        # normalize
                        sum_sb = nrmp.tile([1, 1024], F32, tag="sum")
                        r_sb = nrmp.tile([1, 1024], F32, tag="r")
                        rb_sb = nrmp.tile([64, 1024], F32, tag="rb")
                        nc.vector.tensor_copy(sum_sb[:], at_ps[64:65, :])
                        nc.vector.reciprocal_approx_fast(r_sb[:], sum_sb[:])
                        nc.gpsimd.partition_broadcast(rb_sb[:], r_sb[:])
                        nc.vector.tensor_tensor(
                            at_tiles[pr][r0:r0 + 64, :], at_ps[0:64, :],
                            rb_sb[:], OP.mult,
                        )
                    # projection for this i-half
                    for tl in range(8):
                        tt = 8 * ih + tl
                        pp = stps.tile([128, 1024], F32, tag="st")
                        for nch in range(2):
                            for mt in range(4):
                                nc.tensor.matmul(
                                    pp[:, nch * 512:(nch + 1) * 512],
                                    at_tiles[mt][:, tl * 128:(tl + 1) * 128],
                                    wp_sb[:, mt, nch * 512:(nch + 1) * 512],
                                    start=(mt == 0), stop=(mt == 3),
                                )
                        o_sb = osbp.tile([128, 1024], F32, tag="osb")
                        nc.scalar.copy(o_sb[:], pp[:])
                        nc.sync.dma_start(out_d[tt * 128:(tt + 1) * 128, :], o_sb[:])
    nc.compile()
    return nc


_NC_CACHE = None


def _get_program():
    global _NC_CACHE
    if _NC_CACHE is None:
        _NC_CACHE = _build_program()
    return _NC_CACHE


def _host_inputs(x, cos, sin, w_qkv, w_proj):
    """Build the 8 per-core input dicts."""
    x = np.asarray(x, np.float32)
    cos = np.asarray(cos, np.float32)
    sin = np.asarray(sin, np.float32)
    w_qkv = np.asarray(w_qkv, np.float32)
    w_proj = np.asarray(w_proj, np.float32)

    cct = np.tile(cos.T, (4, 1)).astype(np.float32)          # [128, T]
    sst = np.tile(sin.T, (4, 1)).astype(np.float32)
    tri = np.where(
        np.arange(128)[None, :] >= np.arange(128)[:, None], 1.0, 0.0
    ).astype(np.float32)
    # signed 32-row-block swap (as matmul lhsT): out[m] = s(m) * in[src(m)]
    perm = np.zeros((128, 128), np.float32)
    for m in range(128):
        if m % 64 < 32:
            perm[m + 32, m] = -1.0
        else:
            perm[m - 32, m] = 1.0
    ident = np.eye(128, dtype=np.float32)

    x_r = [_round_fp32r(np.ascontiguousarray(x[b])) for b in range(B)]

    wq = w_qkv[:, 0:D]
    wk = w_qkv[:, D:2 * D] * np.float32(1.0 / np.sqrt(HD))
    wv = w_qkv[:, 2 * D:3 * D]

    def build_qk_aug(g):
        cols = []
        # ft 0..3: q pairs, ft 4..7: k pairs (evens then odds per head)
        for w in (wq, wk):
            for pr in range(4):
                for hl in (0, 1):
                    hw = w[:, (g * 8 + 2 * pr + hl) * 64:(g * 8 + 2 * pr + hl + 1) * 64]
                    ev, od = hw[:, 0::2], hw[:, 1::2]
                    cols.append(np.concatenate([ev, od], axis=1))
        return np.concatenate(cols, axis=1)  # [D, 1024]

    wqk_g = [_round_fp32r(build_qk_aug(g)) for g in range(2)]
    wv_g = [_round_fp32r(np.ascontiguousarray(wv[:, g * 512:(g + 1) * 512]))
            for g in range(2)]
    wp_g = [_round_fp32r(np.ascontiguousarray(w_proj[g * 512:(g + 1) * 512, :]))
            for g in range(2)]

    in_maps = []
    for c in range(N_CORES):
        b, g = c // 2, c % 2
        in_maps.append({
            "x": x_r[b], "wqk": wqk_g[g], "wv": wv_g[g], "wproj": wp_g[g],
            "cc": cct, "ss": sst, "tri": tri, "ident": _round_fp32r(ident),
            "perm": _round_fp32r(perm),
        })
    return in_maps


def kernel(x, cos, sin, w_qkv, w_proj):
    nc = _get_program()
    in_maps = _host_inputs(x, cos, sin, w_qkv, w_proj)
    res = run_bass_kernel_spmd(nc, in_maps, core_ids=list(range(N_CORES)))
    out = np.empty((B, T, D), dtype=np.float32)
    for b in range(B):
        out[b] = res.results[2 * b]["out"] + res.results[2 * b + 1]["out"]
    return out


# revision 14
# speedup vs baseline: 7075.7872x; 6306.8313x over previous
"""Causal self-attention with RoPE on 8 Trainium2 NeuronCores.

Reference computation (B=4, T=2048, D=1024, H=16, hd=64, fp32):
    qkv = x @ w_qkv ; q,k per-head RoPE (interleaved pairs) ;
    out = softmax(causal(q k^T / 8)) @ v ; out @ w_proj

Sharding: core c -> (batch b = c//2, head-group g = c%2 of 8 heads).
Data parallel on B, tensor parallel on heads; w_proj is row-parallel so each
core returns a partial [2048, 1024] product and the host sums the two
partials per batch (the "all-reduce" of the row-parallel linear).

Per-core device program (all matmuls in fp32r: fp32 with 11-bit mantissa
round, full PE speed; accumulation fp32 in PSUM):
  1. x^T via PE transposes (x pre-rounded to fp32r on host).
  2. qkv: q,k in transposed layout [feat, t] (lhsT = w tiles, rhs = x^T);
     v in natural layout [t, feat] (lhsT = x^T tiles, rhs = w_v).
     The q/k weight matrix is augmented host-side with rotated duplicates
     (q' = [-w_odd | w_even] per head) so RoPE becomes
       q_rot = q * cc + q' * ss      (2 DVE mults + 1 add, no partition swap)
     with cc/ss = cos/sin tables replicated across the 4x32 partition rows.
     The 1/sqrt(hd) score scale is folded into the k columns host-side.
  3. Attention per head in transposed-score layout: S^T[j, i] tiles
     (j = keys on partitions), causal mask as additive -3e38 on the diagonal
     128x128 block, exp on ScalarE straight into fp32r P^T tiles, then
     out^T[d, i] accumulated as matmul(lhsT=[v | ones], rhs=P^T) -- the ones
     column yields the softmax denominators in row 64 for free.
     Normalize: reciprocal_approx_fast + gpsimd partition_broadcast + mult.
  4. out_partial = attnT @ w_proj (row-parallel slice), DMA to DRAM.
"""

import numpy as np

import concourse.bass as bass
import concourse.tile as tile
from concourse import bacc, mybir
from concourse.bass_utils import run_bass_kernel_spmd
from neuron_dtypes._impl import fp32r as fp32r_impl

F32 = mybir.dt.float32
F32R = mybir.dt.float32r
AF = mybir.ActivationFunctionType
OP = mybir.AluOpType

B, T, D, NH, HD = 4, 2048, 1024, 16, 64
HPC = 8            # heads per core
NEG = -3.0e38
N_CORES = 8
NTT = T // 128     # 16 token tiles
NKT = D // 128     # 8 contraction tiles


def _round_fp32r(x: np.ndarray) -> np.ndarray:
    xb = np.ascontiguousarray(x).view(np.uint32).ravel()
    r = np.asarray(fp32r_impl.cast_fp32_to_fp32r(len(xb), xb), dtype=np.uint32)
    return r.view(np.float32).reshape(x.shape)


def _build_program(reps: int = 1, stop_after: str = 'full', reps_ab: int = 1, reps_c: int = 1):
    nc = bacc.Bacc("TRN2", target_bir_lowering=False, debug=False)
    x_d = nc.dram_tensor("x", [T, D], F32R, kind="ExternalInput")
    wqk_d = nc.dram_tensor("wqk", [D, 1024], F32R, kind="ExternalInput")
    perm_d = nc.dram_tensor("perm", [128, 128], F32R, kind="ExternalInput")
    wv_d = nc.dram_tensor("wv", [D, 512], F32R, kind="ExternalInput")
    wp_d = nc.dram_tensor("wproj", [512, D], F32R, kind="ExternalInput")
    cc_d = nc.dram_tensor("cc", [128, T], F32, kind="ExternalInput")
    ss_d = nc.dram_tensor("ss", [128, T], F32, kind="ExternalInput")
    tri_d = nc.dram_tensor("tri", [128, 128], F32, kind="ExternalInput")
    id_d = nc.dram_tensor("ident", [128, 128], F32R, kind="ExternalInput")
    out_d = nc.dram_tensor("out", [T, D], F32, kind="ExternalOutput")

    with tile.TileContext(nc) as tc:
      for _rep in range(reps):
        with (
            tc.tile_pool(name="persist", bufs=1) as pers,
            tc.tile_pool(name="vo", bufs=1) as vop,
            tc.tile_pool(name="qkt", bufs=1) as qktp,
        ):
            ident = pers.tile([128, 128], F32R, tag="ident")
            tri = pers.tile([128, 128], F32, tag="tri")
            perm = pers.tile([128, 128], F32R, tag="perm")
            nc.sync.dma_start(ident[:], id_d[:])
            nc.sync.dma_start(tri[:], tri_d[:])
            nc.sync.dma_start(perm[:], perm_d[:])

            # [128, h, 65] per token tile: v columns 0:64, ones at col 64
            vo = [vop.tile([128, HPC, 65], F32R, tag=f"vo{tt}", name=f"vo{tt}") for tt in range(NTT)]

            for _rab in range(reps_ab):
              with (
                tc.tile_pool(name="xt", bufs=1) as xtp,
              ):
                # ---- phase A: x -> x^T --------------------------------------
                xt = [xtp.tile([128, T], F32R, tag=f"xt{kt}", name=f"xt{kt}") for kt in range(NKT)]
                with (
                    tc.tile_pool(name="xnat", bufs=1) as xnp,
                    tc.tile_pool(name="xtps", bufs=4, space="PSUM") as xtps,
                ):
                    for tg in range(NTT // 4):
                        xn = []
                        for tl in range(4):
                            t_ = xnp.tile([128, D], F32R, tag=f"xn{tl}", name=f"xn{tg}_{tl}")
                            nc.sync.dma_start(
                                t_[:], x_d[(tg * 4 + tl) * 128:(tg * 4 + tl + 1) * 128, :]
                            )
                            xn.append(t_)
                        for kt in range(NKT):
                            ps = xtps.tile([128, 512], F32R, tag="xtps")
                            for tl in range(4):
                                nc.tensor.transpose(
                                    ps[:, tl * 128:(tl + 1) * 128],
                                    xn[tl][:, kt * 128:(kt + 1) * 128],
                                    ident[:],
                                )
                            nc.scalar.copy(
                                xt[kt][:, tg * 512:(tg + 1) * 512], ps[:]
                            )

                # ---- phase B: qkv + rope ------------------------------------
                with (
                    tc.tile_pool(name="wvp", bufs=1) as wvp,
                    tc.tile_pool(name="vps", bufs=2, space="PSUM") as vps,
                ):
                    wv_sb = wvp.tile([128, NKT, 512], F32R, tag="wv")
                    nc.sync.dma_start(
                        wv_sb[:], wv_d.rearrange("(t p) f -> p t f", p=128)
                    )

                    # v (natural layout) + ones column
                    for tt in range(NTT):
                        nc.vector.memset(vo[tt][:].bitcast(F32), 1.0)
                        ps = vps.tile([128, 512], F32, tag="vps")
                        for kt in range(NKT):
                            nc.tensor.matmul(
                                ps[:],
                                xt[kt][:, tt * 128:(tt + 1) * 128],
                                wv_sb[:, kt, :],
                                start=(kt == 0), stop=(kt == NKT - 1),
                            )
                        nc.scalar.copy(
                            vo[tt][:, :, 0:64],
                            ps[:].rearrange("p (h d) -> p h d", h=HPC),
                        )

                with (
                    tc.tile_pool(name="ccss", bufs=1) as ccssp,
                    tc.tile_pool(name="ftw", bufs=2) as ftwp,
                    tc.tile_pool(name="ropet", bufs=2) as ropetp,
                    tc.tile_pool(name="qkps", bufs=2, space="PSUM") as qkps,
                ):
                    cc = ccssp.tile([128, T], F32, tag="cc")
                    ss = ccssp.tile([128, T], F32, tag="ss")
                    nc.sync.dma_start(cc[:], cc_d[:])
                    nc.sync.dma_start(ss[:], ss_d[:])

                    # q/k transposed + rope
                    # qkt tiles 0..3 = roped q pairs, 4..7 = roped k pairs
                    qkt = [
                        qktp.tile([128, T], F32R, tag=f"qkt{i}", name=f"qkt{i}")
                        for i in range(8)
                    ]
                    for pr in range(4):            # head pair
                        for side in range(2):      # 0 = q, 1 = k
                            ft = 4 * side + pr
                            w_a = ftwp.tile([128, NKT, 128], F32R, tag="wa")
                            wr = wqk_d.rearrange("(t p) f -> p t f", p=128)
                            nc.sync.dma_start(w_a[:], wr[:, :, ft * 128:(ft + 1) * 128])
                            def emit_qmm(tcn):
                                sl = slice(tcn * 512, (tcn + 1) * 512)
                                ps_a = qkps.tile([128, 512], F32, tag="qkps",
                                                 name=f"qk{side}_{pr}_{tcn}")
                                for kt in range(NKT):
                                    nc.tensor.matmul(
                                        ps_a[:], w_a[:, kt, :], xt[kt][:, sl],
                                        start=(kt == 0), stop=(kt == NKT - 1),
                                    )
                                q_tmp = ropetp.tile([128, 512], F32R, tag="qtmp",
                                                    name=f"qt{side}_{pr}_{tcn}")
                                nc.scalar.copy(q_tmp[:], ps_a[:])
                                return ps_a, q_tmp

                            def emit_rope(tcn, ps_a, q_tmp):
                                sl = slice(tcn * 512, (tcn + 1) * 512)
                                ps_b = qkps.tile([128, 512], F32, tag="qkpsb",
                                                 name=f"qkb{side}_{pr}_{tcn}")
                                # rotated copy: ps_b = signed 32-row-block swap of q
                                nc.tensor.matmul(
                                    ps_b[:], perm[:], q_tmp[:],
                                    start=True, stop=True,
                                )
                                t1 = ropetp.tile([128, 512], F32, tag="t1")
                                t2 = ropetp.tile([128, 512], F32, tag="t2")
                                nc.vector.tensor_tensor(t1[:], ps_a[:], cc[:, sl], OP.mult)
                                nc.vector.tensor_tensor(t2[:], ps_b[:], ss[:, sl], OP.mult)
                                nc.vector.tensor_tensor(
                                    qkt[4 * side + pr][:, sl], t1[:], t2[:], OP.add
                                )

                            prev_c = None
                            for tcn in range(4):
                                cur = emit_qmm(tcn)
                                if prev_c is not None:
                                    emit_rope(prev_c[0], *prev_c[1])
                                prev_c = (tcn, cur)
                            emit_rope(prev_c[0], *prev_c[1])

            if stop_after == 'qkv':
                with tc.tile_pool(name="dump", bufs=2) as dp:
                    for i in range(8):
                        for tcn in range(2):
                            d = dp.tile([128, 1024], F32, tag="d")
                            nc.vector.tensor_copy(d[:], qkt[i][:, tcn*1024:(tcn+1)*1024].bitcast(F32))
                            blk = 2*i + tcn
                            nc.sync.dma_start(out_d[blk*128:(blk+1)*128, :], d[:])
                continue
            # ---- phase C: attention + projection ----------------------------
            for _rc in range(reps_c):
              with (
                tc.tile_pool(name="wpp", bufs=1) as wpp,
                tc.tile_pool(name="atn", bufs=2) as atnp,
                tc.tile_pool(name="pt", bufs=4) as ptp,
                tc.tile_pool(name="nrm", bufs=2) as nrmp,
                tc.tile_pool(name="osb", bufs=2) as osbp,
                tc.tile_pool(name="stps", bufs=2, space="PSUM") as stps,
                tc.tile_pool(name="atps", bufs=2, space="PSUM") as atps,
            ):
                wp_sb = wpp.tile([128, 4, D], F32R, tag="wp")
                nc.sync.dma_start(wp_sb[:], wp_d.rearrange("(t p) f -> p t f", p=128))

                for ih in range(2):
                    i0 = 1024 * ih
                    at_tiles = [
                        atnp.tile([128, 1024], F32R, tag=f"at{pr}", name=f"at{ih}_{pr}")
                        for pr in range(4)
                    ]
                    n_jt = 8 * ih + 8

                    def jt_segs(jt):
                        j0 = 128 * jt
                        i_lo = max(i0, j0)
                        segs = []
                        lo = i_lo
                        while lo < i0 + 1024:
                            hi = min(i0 + 1024, (lo // 512 + 1) * 512)
                            segs.append((lo, hi))
                            lo = hi
                        return j0, i_lo, segs

                    for pr in range(4):
                        # two heads of the pair interleaved: even head uses PE
                        # rows 0:64, odd head rows 64:128 -> their K=64 score
                        # matmuls run concurrently on disjoint PE quadrants
                        heads = (2 * pr, 2 * pr + 1)
                        at_ps_h = {}
                        for h in heads:
                            at_ps_h[h] = atps.tile(
                                [65, 1024], F32, tag="atps", name=f"at{ih}_{h}"
                            )

                        def emit_st(h, jt):
                            r0 = 64 * (h % 2)
                            qt_ap = qkt[pr][r0:r0 + 64, :]
                            kt_ap = qkt[4 + pr][r0:r0 + 64, :]
                            j0, i_lo, segs = jt_segs(jt)
                            st = stps.tile([128, 1024], F32, tag="st",
                                           name=f"st{ih}_{h}_{jt}")
                            for (lo, hi) in segs:
                                nc.tensor.matmul(
                                    st[:, lo - i0:hi - i0],
                                    kt_ap[:, j0:j0 + 128],
                                    qt_ap[:, lo:hi],
                                    start=True, stop=True,
                                )
                            if j0 >= i0:
                                nc.vector.tensor_tensor(
                                    st[:, j0 - i0:j0 - i0 + 128],
                                    st[:, j0 - i0:j0 - i0 + 128],
                                    tri[:], OP.add,
                                )
                            pt = ptp.tile([128, 1024], F32R, tag="pt",
                                          name=f"pt{ih}_{h}_{jt}")
                            nc.scalar.activation(
                                pt[:, 0:i0 + 1024 - i_lo], st[:, i_lo - i0:], AF.Exp
                            )
                            return pt

                        def emit_pv(h, jt, pt):
                            j0, i_lo, segs = jt_segs(jt)
                            for (lo, hi) in segs:
                                last_jt = min(n_jt - 1, (hi - 1) // 128)
                                nc.tensor.matmul(
                                    at_ps_h[h][:, lo - i0:hi - i0],
                                    vo[jt][:, h, :],
                                    pt[:, lo - i_lo:hi - i_lo],
                                    start=(jt == 0), stop=(jt == last_jt),
                                )

                        prev = {h: None for h in heads}
                        for jt in range(n_jt):
                            pts = {}
                            for h in heads:
                                pts[h] = emit_st(h, jt)
                            for h in heads:
                                if prev[h] is not None:
                                    emit_pv(h, jt - 1, prev[h])
                                prev[h] = pts[h]
                        for h in heads:
                            emit_pv(h, n_jt - 1, prev[h])

                        for h in heads:
                            r0 = 64 * (h % 2)
                            sum_sb = nrmp.tile([1, 1024], F32, tag="sum")
                            r_sb = nrmp.tile([1, 1024], F32, tag="r")
                            rb_sb = nrmp.tile([64, 1024], F32, tag="rb")
                            nc.vector.tensor_copy(sum_sb[:], at_ps_h[h][64:65, :])
                            nc.vector.reciprocal_approx_fast(r_sb[:], sum_sb[:])
                            nc.gpsimd.partition_broadcast(rb_sb[:], r_sb[:])
                            nc.vector.tensor_tensor(
                                at_tiles[pr][r0:r0 + 64, :], at_ps_h[h][0:64, :],
                                rb_sb[:], OP.mult,
                            )
                    # projection for this i-half
                    for tl in range(8 if stop_after == 'full' else 0):
                        tt = 8 * ih + tl
                        pp = stps.tile([128, 1024], F32, tag="st")
                        for nch in range(2):
                            for mt in range(4):
                                nc.tensor.matmul(
                                    pp[:, nch * 512:(nch + 1) * 512],
                                    at_tiles[mt][:, tl * 128:(tl + 1) * 128],
                                    wp_sb[:, mt, nch * 512:(nch + 1) * 512],
                                    start=(mt == 0), stop=(mt == 3),
                                )
                        o_sb = osbp.tile([128, 1024], F32, tag="osb")
                        nc.scalar.copy(o_sb[:], pp[:])
                        nc.sync.dma_start(out_d[tt * 128:(tt + 1) * 128, :], o_sb[:])
                    if stop_after == 'attn':
                        for pr in range(4):
                            d = osbp.tile([128, 1024], F32, tag="osb")
                            nc.vector.tensor_copy(d[:], at_tiles[pr][:].bitcast(F32))
                            nc.sync.dma_start(
                                out_d[(8*ih+2*pr)*128:(8*ih+2*pr+1)*128, :], d[:])
    nc.compile()
    return nc


_NC_CACHE = None


def _get_program():
    global _NC_CACHE
    if _NC_CACHE is None:
        _NC_CACHE = _build_program()
    return _NC_CACHE


def _host_inputs(x, cos, sin, w_qkv, w_proj):
    """Build the 8 per-core input dicts."""
    x = np.asarray(x, np.float32)
    cos = np.asarray(cos, np.float32)
    sin = np.asarray(sin, np.float32)
    w_qkv = np.asarray(w_qkv, np.float32)
    w_proj = np.asarray(w_proj, np.float32)

    cct = np.tile(cos.T, (4, 1)).astype(np.float32)          # [128, T]
    sst = np.tile(sin.T, (4, 1)).astype(np.float32)
    tri = np.where(
        np.arange(128)[None, :] >= np.arange(128)[:, None], 0.0, NEG
    ).astype(np.float32)
    # signed 32-row-block swap (as matmul lhsT): out[m] = s(m) * in[src(m)]
    perm = np.zeros((128, 128), np.float32)
    for m in range(128):
        if m % 64 < 32:
            perm[m + 32, m] = -1.0
        else:
            perm[m - 32, m] = 1.0
    ident = np.eye(128, dtype=np.float32)

    x_r = [_round_fp32r(np.ascontiguousarray(x[b])) for b in range(B)]

    wq = w_qkv[:, 0:D]
    wk = w_qkv[:, D:2 * D] * np.float32(1.0 / np.sqrt(HD))
    wv = w_qkv[:, 2 * D:3 * D]

    def build_qk_aug(g):
        cols = []
        # ft 0..3: q pairs, ft 4..7: k pairs (evens then odds per head)
        for w in (wq, wk):
            for pr in range(4):
                for hl in (0, 1):
                    hw = w[:, (g * 8 + 2 * pr + hl) * 64:(g * 8 + 2 * pr + hl + 1) * 64]
                    ev, od = hw[:, 0::2], hw[:, 1::2]
                    cols.append(np.concatenate([ev, od], axis=1))
        return np.concatenate(cols, axis=1)  # [D, 1024]

    wqk_g = [_round_fp32r(build_qk_aug(g)) for g in range(2)]
    wv_g = [_round_fp32r(np.ascontiguousarray(wv[:, g * 512:(g + 1) * 512]))
            for g in range(2)]
    wp_g = [_round_fp32r(np.ascontiguousarray(w_proj[g * 512:(g + 1) * 512, :]))
            for g in range(2)]

    in_maps = []
    for c in range(N_CORES):
        b, g = c // 2, c % 2
        in_maps.append({
            "x": x_r[b], "wqk": wqk_g[g], "wv": wv_g[g], "wproj": wp_g[g],
            "cc": cct, "ss": sst, "tri": tri, "ident": _round_fp32r(ident),
            "perm": _round_fp32r(perm),
        })
    return in_maps


def kernel(x, cos, sin, w_qkv, w_proj):
    nc = _get_program()
    in_maps = _host_inputs(x, cos, sin, w_qkv, w_proj)
    res = run_bass_kernel_spmd(nc, in_maps, core_ids=list(range(N_CORES)))
    out = np.empty((B, T, D), dtype=np.float32)
    for b in range(B):
        out[b] = res.results[2 * b]["out"] + res.results[2 * b + 1]["out"]
    return out


# revision 15
# speedup vs baseline: 9656.0587x; 1.3647x over previous
"""Causal self-attention with RoPE on 8 Trainium2 NeuronCores.

Reference computation (B=4, T=2048, D=1024, H=16, hd=64, fp32):
    qkv = x @ w_qkv ; q,k per-head RoPE (interleaved pairs) ;
    out = softmax(causal(q k^T / 8)) @ v ; out @ w_proj

Sharding: core c -> (batch b = c//2, head-group g = c%2 of 8 heads).
Data parallel on B, tensor parallel on heads; w_proj is row-parallel so each
core returns a partial [2048, 1024] product and the host sums the two
partials per batch (the "all-reduce" of the row-parallel linear).

Per-core device program (all matmuls in fp32r: fp32 with 11-bit mantissa
round, full PE speed; accumulation fp32 in PSUM):
  1. x^T via PE transposes (x pre-rounded to fp32r on host).
  2. qkv: q,k in transposed layout [feat, t] (lhsT = w tiles, rhs = x^T);
     v in natural layout [t, feat] (lhsT = x^T tiles, rhs = w_v).
     The q/k weight matrix is augmented host-side with rotated duplicates
     (q' = [-w_odd | w_even] per head) so RoPE becomes
       q_rot = q * cc + q' * ss      (2 DVE mults + 1 add, no partition swap)
     with cc/ss = cos/sin tables replicated across the 4x32 partition rows.
     The 1/sqrt(hd) score scale is folded into the k columns host-side.
  3. Attention per head in transposed-score layout: S^T[j, i] tiles
     (j = keys on partitions), causal mask as additive -3e38 on the diagonal
     128x128 block, exp on ScalarE straight into fp32r P^T tiles, then
     out^T[d, i] accumulated as matmul(lhsT=[v | ones], rhs=P^T) -- the ones
     column yields the softmax denominators in row 64 for free.
     Normalize: reciprocal_approx_fast + gpsimd partition_broadcast + mult.
  4. out_partial = attnT @ w_proj (row-parallel slice), DMA to DRAM.
"""

import numpy as np

import concourse.bass as bass
import concourse.tile as tile
from concourse import bacc, mybir
from concourse.bass_utils import run_bass_kernel_spmd
from neuron_dtypes._impl import fp32r as fp32r_impl

F32 = mybir.dt.float32
F32R = mybir.dt.float32r
AF = mybir.ActivationFunctionType
OP = mybir.AluOpType

B, T, D, NH, HD = 4, 2048, 1024, 16, 64
HPC = 8            # heads per core
NEG = -3.0e38
N_CORES = 8
NTT = T // 128     # 16 token tiles
NKT = D // 128     # 8 contraction tiles


def _round_fp32r(x: np.ndarray) -> np.ndarray:
    xb = np.ascontiguousarray(x).view(np.uint32).ravel()
    r = np.asarray(fp32r_impl.cast_fp32_to_fp32r(len(xb), xb), dtype=np.uint32)
    return r.view(np.float32).reshape(x.shape)


def _build_program(reps: int = 1, stop_after: str = 'full', reps_ab: int = 1, reps_c: int = 1):
    nc = bacc.Bacc("TRN2", target_bir_lowering=False, debug=False)
    x_d = nc.dram_tensor("x", [T, D], F32R, kind="ExternalInput")
    wqk_d = nc.dram_tensor("wqk", [D, 1024], F32R, kind="ExternalInput")
    perm_d = nc.dram_tensor("perm", [128, 128], F32R, kind="ExternalInput")
    wv_d = nc.dram_tensor("wv", [D, 512], F32R, kind="ExternalInput")
    wp_d = nc.dram_tensor("wproj", [512, D], F32R, kind="ExternalInput")
    cc_d = nc.dram_tensor("cc", [128, T], F32, kind="ExternalInput")
    ss_d = nc.dram_tensor("ss", [128, T], F32, kind="ExternalInput")
    tri_d = nc.dram_tensor("tri", [128, 128], F32, kind="ExternalInput")
    id_d = nc.dram_tensor("ident", [128, 128], F32R, kind="ExternalInput")
    out_d = nc.dram_tensor("out", [T, D], F32, kind="ExternalOutput")

    with tile.TileContext(nc) as tc:
      for _rep in range(reps):
        with (
            tc.tile_pool(name="persist", bufs=1) as pers,
            tc.tile_pool(name="vo", bufs=1) as vop,
            tc.tile_pool(name="qkt", bufs=1) as qktp,
        ):
            ident = pers.tile([128, 128], F32R, tag="ident")
            tri = pers.tile([128, 128], F32, tag="tri")
            perm = pers.tile([128, 128], F32R, tag="perm")
            nc.sync.dma_start(ident[:], id_d[:])
            nc.sync.dma_start(tri[:], tri_d[:])
            nc.sync.dma_start(perm[:], perm_d[:])

            # [128, h, 65] per token tile: v columns 0:64, ones at col 64
            vo = [vop.tile([128, HPC, 65], F32R, tag=f"vo{tt}", name=f"vo{tt}") for tt in range(NTT)]

            for _rab in range(reps_ab):
              with (
                tc.tile_pool(name="xt", bufs=1) as xtp,
              ):
                # ---- phase A: x -> x^T --------------------------------------
                xt = [xtp.tile([128, T], F32R, tag=f"xt{kt}", name=f"xt{kt}") for kt in range(NKT)]
                with (
                    tc.tile_pool(name="xnat", bufs=1) as xnp,
                    tc.tile_pool(name="xtps", bufs=4, space="PSUM") as xtps,
                ):
                    for tg in range(NTT // 4):
                        xn = []
                        for tl in range(4):
                            t_ = xnp.tile([128, D], F32R, tag=f"xn{tl}", name=f"xn{tg}_{tl}")
                            nc.sync.dma_start(
                                t_[:], x_d[(tg * 4 + tl) * 128:(tg * 4 + tl + 1) * 128, :]
                            )
                            xn.append(t_)
                        for kt in range(NKT):
                            ps = xtps.tile([128, 512], F32R, tag="xtps")
                            for tl in range(4):
                                nc.tensor.transpose(
                                    ps[:, tl * 128:(tl + 1) * 128],
                                    xn[tl][:, kt * 128:(kt + 1) * 128],
                                    ident[:],
                                )
                            nc.scalar.copy(
                                xt[kt][:, tg * 512:(tg + 1) * 512], ps[:]
                            )

                # ---- phase B: qkv + rope ------------------------------------
                with (
                    tc.tile_pool(name="wvp", bufs=1) as wvp,
                    tc.tile_pool(name="vps", bufs=2, space="PSUM") as vps,
                ):
                    wv_sb = wvp.tile([128, NKT, 512], F32R, tag="wv")
                    nc.sync.dma_start(
                        wv_sb[:], wv_d.rearrange("(t p) f -> p t f", p=128)
                    )

                    # v (natural layout) + ones column
                    for tt in range(NTT):
                        nc.vector.memset(vo[tt][:].bitcast(F32), 1.0)
                        ps = vps.tile([128, 512], F32, tag="vps")
                        for kt in range(NKT):
                            nc.tensor.matmul(
                                ps[:],
                                xt[kt][:, tt * 128:(tt + 1) * 128],
                                wv_sb[:, kt, :],
                                start=(kt == 0), stop=(kt == NKT - 1),
                            )
                        nc.scalar.copy(
                            vo[tt][:, :, 0:64],
                            ps[:].rearrange("p (h d) -> p h d", h=HPC),
                        )

                with (
                    tc.tile_pool(name="ccss", bufs=1) as ccssp,
                    tc.tile_pool(name="ftw", bufs=2) as ftwp,
                    tc.tile_pool(name="ropet", bufs=2) as ropetp,
                    tc.tile_pool(name="qkps", bufs=2, space="PSUM") as qkps,
                ):
                    cc = ccssp.tile([128, T], F32, tag="cc")
                    ss = ccssp.tile([128, T], F32, tag="ss")
                    nc.sync.dma_start(cc[:], cc_d[:])
                    nc.sync.dma_start(ss[:], ss_d[:])

                    # q/k transposed + rope
                    # qkt tiles 0..3 = roped q pairs, 4..7 = roped k pairs
                    qkt = [
                        qktp.tile([128, T], F32R, tag=f"qkt{i}", name=f"qkt{i}")
                        for i in range(8)
                    ]
                    for pr in range(4):            # head pair
                        for side in range(2):      # 0 = q, 1 = k
                            ft = 4 * side + pr
                            w_a = ftwp.tile([128, NKT, 128], F32R, tag="wa")
                            wr = wqk_d.rearrange("(t p) f -> p t f", p=128)
                            nc.sync.dma_start(w_a[:], wr[:, :, ft * 128:(ft + 1) * 128])
                            def emit_qmm(tcn):
                                sl = slice(tcn * 512, (tcn + 1) * 512)
                                ps_a = qkps.tile([128, 512], F32, tag="qkps",
                                                 name=f"qk{side}_{pr}_{tcn}")
                                for kt in range(NKT):
                                    nc.tensor.matmul(
                                        ps_a[:], w_a[:, kt, :], xt[kt][:, sl],
                                        start=(kt == 0), stop=(kt == NKT - 1),
                                    )
                                q_tmp = ropetp.tile([128, 512], F32R, tag="qtmp",
                                                    name=f"qt{side}_{pr}_{tcn}")
                                nc.scalar.copy(q_tmp[:], ps_a[:])
                                return ps_a, q_tmp

                            def emit_rope(tcn, ps_a, q_tmp):
                                sl = slice(tcn * 512, (tcn + 1) * 512)
                                ps_b = qkps.tile([128, 512], F32, tag="qkpsb",
                                                 name=f"qkb{side}_{pr}_{tcn}")
                                # rotated copy: ps_b = signed 32-row-block swap of q
                                nc.tensor.matmul(
                                    ps_b[:], perm[:], q_tmp[:],
                                    start=True, stop=True,
                                )
                                t1 = ropetp.tile([128, 512], F32, tag="t1")
                                t2 = ropetp.tile([128, 512], F32, tag="t2")
                                nc.vector.tensor_tensor(t1[:], ps_a[:], cc[:, sl], OP.mult)
                                nc.vector.tensor_tensor(t2[:], ps_b[:], ss[:, sl], OP.mult)
                                nc.vector.tensor_tensor(
                                    qkt[4 * side + pr][:, sl], t1[:], t2[:], OP.add
                                )

                            prev_c = None
                            for tcn in range(4):
                                cur = emit_qmm(tcn)
                                if prev_c is not None:
                                    emit_rope(prev_c[0], *prev_c[1])
                                prev_c = (tcn, cur)
                            emit_rope(prev_c[0], *prev_c[1])

            if stop_after == 'qkv':
                with tc.tile_pool(name="dump", bufs=2) as dp:
                    for i in range(8):
                        for tcn in range(2):
                            d = dp.tile([128, 1024], F32, tag="d")
                            nc.vector.tensor_copy(d[:], qkt[i][:, tcn*1024:(tcn+1)*1024].bitcast(F32))
                            blk = 2*i + tcn
                            nc.sync.dma_start(out_d[blk*128:(blk+1)*128, :], d[:])
                continue
            # ---- phase C: attention + projection ----------------------------
            for _rc in range(reps_c):
              with (
                tc.tile_pool(name="wpp", bufs=1) as wpp,
                tc.tile_pool(name="atn", bufs=2) as atnp,
                tc.tile_pool(name="pt", bufs=4) as ptp,
                tc.tile_pool(name="nrm", bufs=2) as nrmp,
                tc.tile_pool(name="osb", bufs=2) as osbp,
                tc.tile_pool(name="stps", bufs=2, space="PSUM") as stps,
                tc.tile_pool(name="atps", bufs=2, space="PSUM") as atps,
            ):
                wp_sb = wpp.tile([128, 4, D], F32R, tag="wp")
                nc.sync.dma_start(wp_sb[:], wp_d.rearrange("(t p) f -> p t f", p=128))

                at_by_ih = {}

                def emit_proj(ih2):
                    tiles = at_by_ih[ih2]
                    for tl in range(8 if stop_after == 'full' else 0):
                        tt = 8 * ih2 + tl
                        pp = stps.tile([128, 1024], F32, tag="st")
                        for nch in range(2):
                            for mt in range(4):
                                nc.tensor.matmul(
                                    pp[:, nch * 512:(nch + 1) * 512],
                                    tiles[mt][:, tl * 128:(tl + 1) * 128],
                                    wp_sb[:, mt, nch * 512:(nch + 1) * 512],
                                    start=(mt == 0), stop=(mt == 3),
                                )
                        o_sb = osbp.tile([128, 1024], F32, tag="osb")
                        nc.scalar.copy(o_sb[:], pp[:])
                        nc.sync.dma_start(out_d[tt * 128:(tt + 1) * 128, :], o_sb[:])

                for ih in range(2):
                    i0 = 1024 * ih
                    at_tiles = [
                        atnp.tile([128, 1024], F32R, tag=f"at{pr}", name=f"at{ih}_{pr}")
                        for pr in range(4)
                    ]
                    at_by_ih[ih] = at_tiles
                    n_jt = 8 * ih + 8

                    def jt_segs(jt):
                        j0 = 128 * jt
                        i_lo = max(i0, j0)
                        segs = []
                        lo = i_lo
                        while lo < i0 + 1024:
                            hi = min(i0 + 1024, (lo // 512 + 1) * 512)
                            segs.append((lo, hi))
                            lo = hi
                        return j0, i_lo, segs

                    for pr in range(4):
                        # two heads of the pair interleaved: even head uses PE
                        # rows 0:64, odd head rows 64:128 -> their K=64 score
                        # matmuls run concurrently on disjoint PE quadrants
                        heads = (2 * pr, 2 * pr + 1)
                        at_ps_h = {}
                        for h in heads:
                            at_ps_h[h] = atps.tile(
                                [65, 1024], F32, tag="atps", name=f"at{ih}_{h}"
                            )

                        def emit_st(h, jt):
                            r0 = 64 * (h % 2)
                            qt_ap = qkt[pr][r0:r0 + 64, :]
                            kt_ap = qkt[4 + pr][r0:r0 + 64, :]
                            j0, i_lo, segs = jt_segs(jt)
                            st = stps.tile([128, 1024], F32, tag="st",
                                           name=f"st{ih}_{h}_{jt}")
                            for (lo, hi) in segs:
                                nc.tensor.matmul(
                                    st[:, lo - i0:hi - i0],
                                    kt_ap[:, j0:j0 + 128],
                                    qt_ap[:, lo:hi],
                                    start=True, stop=True,
                                )
                            if j0 >= i0:
                                nc.vector.tensor_tensor(
                                    st[:, j0 - i0:j0 - i0 + 128],
                                    st[:, j0 - i0:j0 - i0 + 128],
                                    tri[:], OP.add,
                                )
                            pt = ptp.tile([128, 1024], F32R, tag="pt",
                                          name=f"pt{ih}_{h}_{jt}")
                            nc.scalar.activation(
                                pt[:, 0:i0 + 1024 - i_lo], st[:, i_lo - i0:], AF.Exp
                            )
                            return pt

                        def emit_pv(h, jt, pt):
                            j0, i_lo, segs = jt_segs(jt)
                            for (lo, hi) in segs:
                                last_jt = min(n_jt - 1, (hi - 1) // 128)
                                nc.tensor.matmul(
                                    at_ps_h[h][:, lo - i0:hi - i0],
                                    vo[jt][:, h, :],
                                    pt[:, lo - i_lo:hi - i_lo],
                                    start=(jt == 0), stop=(jt == last_jt),
                                )

                        prev = {h: None for h in heads}
                        for jt in range(n_jt):
                            pts = {}
                            for h in heads:
                                pts[h] = emit_st(h, jt)
                            for h in heads:
                                if prev[h] is not None:
                                    emit_pv(h, jt - 1, prev[h])
                                prev[h] = pts[h]
                        for h in heads:
                            emit_pv(h, n_jt - 1, prev[h])

                        for h in heads:
                            r0 = 64 * (h % 2)
                            sum_sb = nrmp.tile([1, 1024], F32, tag="sum")
                            r_sb = nrmp.tile([1, 1024], F32, tag="r")
                            rb_sb = nrmp.tile([64, 1024], F32, tag="rb")
                            nc.vector.tensor_copy(sum_sb[:], at_ps_h[h][64:65, :])
                            nc.vector.reciprocal_approx_fast(r_sb[:], sum_sb[:])
                            nc.gpsimd.partition_broadcast(rb_sb[:], r_sb[:])
                            nc.vector.tensor_tensor(
                                at_tiles[pr][r0:r0 + 64, :], at_ps_h[h][0:64, :],
                                rb_sb[:], OP.mult,
                            )
                        if ih == 1 and pr == 0:
                            emit_proj(0)
                    # proj(0) is emitted inside ih=1 (after its first head
                    # pair) so its matmuls fill PE idle time under the
                    # ACT-bound second-half attention; proj(1) at the end.
                    if ih == 1:
                        emit_proj(1)
                    if stop_after == 'attn':
                        for pr in range(4):
                            d = osbp.tile([128, 1024], F32, tag="osb")
                            nc.vector.tensor_copy(d[:], at_tiles[pr][:].bitcast(F32))
                            nc.sync.dma_start(
                                out_d[(8*ih+2*pr)*128:(8*ih+2*pr+1)*128, :], d[:])
    nc.compile()
    return nc


_NC_CACHE = None


def _get_program():
    global _NC_CACHE
    if _NC_CACHE is None:
        _NC_CACHE = _build_program()
    return _NC_CACHE


def _host_inputs(x, cos, sin, w_qkv, w_proj):
    """Build the 8 per-core input dicts."""
    x = np.asarray(x, np.float32)
    cos = np.asarray(cos, np.float32)
    sin = np.asarray(sin, np.float32)
    w_qkv = np.asarray(w_qkv, np.float32)
    w_proj = np.asarray(w_proj, np.float32)

    cct = np.tile(cos.T, (4, 1)).astype(np.float32)          # [128, T]
    sst = np.tile(sin.T, (4, 1)).astype(np.float32)
    tri = np.where(
        np.arange(128)[None, :] >= np.arange(128)[:, None], 0.0, NEG
    ).astype(np.float32)
    # signed 32-row-block swap (as matmul lhsT): out[m] = s(m) * in[src(m)]
    perm = np.zeros((128, 128), np.float32)
    for m in range(128):
        if m % 64 < 32:
            perm[m + 32, m] = -1.0
        else:
            perm[m - 32, m] = 1.0
    ident = np.eye(128, dtype=np.float32)

    x_r = [_round_fp32r(np.ascontiguousarray(x[b])) for b in range(B)]

    wq = w_qkv[:, 0:D]
    wk = w_qkv[:, D:2 * D] * np.float32(1.0 / np.sqrt(HD))
    wv = w_qkv[:, 2 * D:3 * D]

    def build_qk_aug(g):
        cols = []
        # ft 0..3: q pairs, ft 4..7: k pairs (evens then odds per head)
        for w in (wq, wk):
            for pr in range(4):
                for hl in (0, 1):
                    hw = w[:, (g * 8 + 2 * pr + hl) * 64:(g * 8 + 2 * pr + hl + 1) * 64]
                    ev, od = hw[:, 0::2], hw[:, 1::2]
                    cols.append(np.concatenate([ev, od], axis=1))
        return np.concatenate(cols, axis=1)  # [D, 1024]

    wqk_g = [_round_fp32r(build_qk_aug(g)) for g in range(2)]
    wv_g = [_round_fp32r(np.ascontiguousarray(wv[:, g * 512:(g + 1) * 512]))
            for g in range(2)]
    wp_g = [_round_fp32r(np.ascontiguousarray(w_proj[g * 512:(g + 1) * 512, :]))
            for g in range(2)]

    in_maps = []
    for c in range(N_CORES):
        b, g = c // 2, c % 2
        in_maps.append({
            "x": x_r[b], "wqk": wqk_g[g], "wv": wv_g[g], "wproj": wp_g[g],
            "cc": cct, "ss": sst, "tri": tri, "ident": _round_fp32r(ident),
            "perm": _round_fp32r(perm),
        })
    return in_maps


def kernel(x, cos, sin, w_qkv, w_proj):
    nc = _get_program()
    in_maps = _host_inputs(x, cos, sin, w_qkv, w_proj)
    res = run_bass_kernel_spmd(nc, in_maps, core_ids=list(range(N_CORES)))
    out = np.empty((B, T, D), dtype=np.float32)
    for b in range(B):
        out[b] = res.results[2 * b]["out"] + res.results[2 * b + 1]["out"]
    return out


# revision 20
# speedup vs baseline: 9799.0730x; 1.0148x over previous
"""Causal self-attention with RoPE on 8 Trainium2 NeuronCores.

Reference computation (B=4, T=2048, D=1024, H=16, hd=64, fp32):
    qkv = x @ w_qkv ; q,k per-head RoPE (interleaved pairs) ;
    out = softmax(causal(q k^T / 8)) @ v ; out @ w_proj

Sharding: core c -> (batch b = c//2, head-group g = c%2 of 8 heads).
Data parallel on B, tensor parallel on heads; w_proj is row-parallel so each
core returns a partial [2048, 1024] product and the host sums the two
partials per batch (the "all-reduce" of the row-parallel linear).

Per-core device program (all matmuls in fp32r: fp32 with 11-bit mantissa
round, full PE speed; accumulation fp32 in PSUM):
  1. x^T via PE transposes (x pre-rounded to fp32r on host).
  2. qkv: q,k in transposed layout [feat, t] (lhsT = w tiles, rhs = x^T);
     v in natural layout [t, feat] (lhsT = x^T tiles, rhs = w_v).
     The q/k weight matrix is augmented host-side with rotated duplicates
     (q' = [-w_odd | w_even] per head) so RoPE becomes
       q_rot = q * cc + q' * ss      (2 DVE mults + 1 add, no partition swap)
     with cc/ss = cos/sin tables replicated across the 4x32 partition rows.
     The 1/sqrt(hd) score scale is folded into the k columns host-side.
  3. Attention per head in transposed-score layout: S^T[j, i] tiles
     (j = keys on partitions), causal mask as additive -3e38 on the diagonal
     128x128 block, exp on ScalarE straight into fp32r P^T tiles, then
     out^T[d, i] accumulated as matmul(lhsT=[v | ones], rhs=P^T) -- the ones
     column yields the softmax denominators in row 64 for free.
     Normalize: reciprocal_approx_fast + gpsimd partition_broadcast + mult.
  4. out_partial = attnT @ w_proj (row-parallel slice), DMA to DRAM.
"""

import numpy as np

import concourse.bass as bass
import concourse.tile as tile
from concourse import bacc, mybir
from concourse.bass_utils import run_bass_kernel_spmd
from neuron_dtypes._impl import fp32r as fp32r_impl

F32 = mybir.dt.float32
F32R = mybir.dt.float32r
AF = mybir.ActivationFunctionType
OP = mybir.AluOpType

B, T, D, NH, HD = 4, 2048, 1024, 16, 64
HPC = 8            # heads per core
NEG = -3.0e38
N_CORES = 8
NTT = T // 128     # 16 token tiles
NKT = D // 128     # 8 contraction tiles


def _round_fp32r(x: np.ndarray) -> np.ndarray:
    xb = np.ascontiguousarray(x).view(np.uint32).ravel()
    r = np.asarray(fp32r_impl.cast_fp32_to_fp32r(len(xb), xb), dtype=np.uint32)
    return r.view(np.float32).reshape(x.shape)


def _build_program(reps: int = 1, stop_after: str = 'full', reps_ab: int = 1, reps_c: int = 1):
    nc = bacc.Bacc("TRN2", target_bir_lowering=False, debug=False)
    x_d = nc.dram_tensor("x", [T, D], F32R, kind="ExternalInput")
    wqk_d = nc.dram_tensor("wqk", [D, 1024], F32R, kind="ExternalInput")
    perm_d = nc.dram_tensor("perm", [128, 128], F32R, kind="ExternalInput")
    wv_d = nc.dram_tensor("wv", [D, 512], F32R, kind="ExternalInput")
    wp_d = nc.dram_tensor("wproj", [512, D], F32R, kind="ExternalInput")
    cc_d = nc.dram_tensor("cc", [128, T], F32, kind="ExternalInput")
    ss_d = nc.dram_tensor("ss", [128, T], F32, kind="ExternalInput")
    tri_d = nc.dram_tensor("tri", [128, 128], F32, kind="ExternalInput")
    id_d = nc.dram_tensor("ident", [128, 128], F32R, kind="ExternalInput")
    out_d = nc.dram_tensor("out", [T, D], F32, kind="ExternalOutput")

    with tile.TileContext(nc) as tc:
      for _rep in range(reps):
        with (
            tc.tile_pool(name="persist", bufs=1) as pers,
            tc.tile_pool(name="vo", bufs=1) as vop,
            tc.tile_pool(name="qkt", bufs=1) as qktp,
        ):
            ident = pers.tile([128, 128], F32R, tag="ident")
            tri = pers.tile([128, 128], F32, tag="tri")
            perm = pers.tile([128, 128], F32R, tag="perm")
            nc.sync.dma_start(ident[:], id_d[:])
            nc.sync.dma_start(tri[:], tri_d[:])
            nc.sync.dma_start(perm[:], perm_d[:])

            # [128, h, 65] per token tile: v columns 0:64, ones at col 64
            vo = [vop.tile([128, HPC, 65], F32R, tag=f"vo{tt}", name=f"vo{tt}") for tt in range(NTT)]

            for _rab in range(reps_ab):
              with (
                tc.tile_pool(name="xt", bufs=1) as xtp,
              ):
                # ---- phase A: x -> x^T --------------------------------------
                xt = [xtp.tile([128, T], F32R, tag=f"xt{kt}", name=f"xt{kt}") for kt in range(NKT)]
                with (
                    tc.tile_pool(name="xnat", bufs=2) as xnp,
                    tc.tile_pool(name="xtps", bufs=4, space="PSUM") as xtps,
                ):
                    for tg in range(NTT // 4):
                        xn = []
                        for tl in range(4):
                            t_ = xnp.tile([128, D], F32R, tag=f"xn{tl}", name=f"xn{tg}_{tl}")
                            nc.sync.dma_start(
                                t_[:], x_d[(tg * 4 + tl) * 128:(tg * 4 + tl + 1) * 128, :]
                            )
                            xn.append(t_)
                        for kt in range(NKT):
                            ps = xtps.tile([128, 512], F32R, tag="xtps")
                            for tl in range(4):
                                nc.tensor.transpose(
                                    ps[:, tl * 128:(tl + 1) * 128],
                                    xn[tl][:, kt * 128:(kt + 1) * 128],
                                    ident[:],
                                )
                            nc.scalar.copy(
                                xt[kt][:, tg * 512:(tg + 1) * 512], ps[:]
                            )

                # ---- phase B: qkv + rope ------------------------------------
                with (
                    tc.tile_pool(name="wvp", bufs=1) as wvp,
                    tc.tile_pool(name="vps", bufs=2, space="PSUM") as vps,
                ):
                    wv_sb = wvp.tile([128, NKT, 512], F32R, tag="wv")
                    nc.sync.dma_start(
                        wv_sb[:], wv_d.rearrange("(t p) f -> p t f", p=128)
                    )

                    # v (natural layout) + ones column
                    for tt in range(NTT):
                        nc.vector.memset(vo[tt][:].bitcast(F32), 1.0)
                        ps = vps.tile([128, 512], F32, tag="vps")
                        for kt in range(NKT):
                            nc.tensor.matmul(
                                ps[:],
                                xt[kt][:, tt * 128:(tt + 1) * 128],
                                wv_sb[:, kt, :],
                                start=(kt == 0), stop=(kt == NKT - 1),
                            )
                        nc.scalar.copy(
                            vo[tt][:, :, 0:64],
                            ps[:].rearrange("p (h d) -> p h d", h=HPC),
                        )

                with (
                    tc.tile_pool(name="ccss", bufs=1) as ccssp,
                    tc.tile_pool(name="ftw", bufs=2) as ftwp,
                    tc.tile_pool(name="ropet", bufs=2) as ropetp,
                    tc.tile_pool(name="qkps", bufs=2, space="PSUM") as qkps,
                ):
                    cc = ccssp.tile([128, T], F32, tag="cc")
                    ss = ccssp.tile([128, T], F32, tag="ss")
                    nc.sync.dma_start(cc[:], cc_d[:])
                    nc.sync.dma_start(ss[:], ss_d[:])

                    # q/k transposed + rope
                    # qkt tiles 0..3 = roped q pairs, 4..7 = roped k pairs
                    qkt = [
                        qktp.tile([128, T], F32R, tag=f"qkt{i}", name=f"qkt{i}")
                        for i in range(8)
                    ]
                    for pr in range(4):            # head pair
                        for side in range(2):      # 0 = q, 1 = k
                            ft = 4 * side + pr
                            w_a = ftwp.tile([128, NKT, 128], F32R, tag="wa")
                            wr = wqk_d.rearrange("(t p) f -> p t f", p=128)
                            nc.sync.dma_start(w_a[:], wr[:, :, ft * 128:(ft + 1) * 128])
                            def emit_qmm(tcn):
                                sl = slice(tcn * 512, (tcn + 1) * 512)
                                ps_a = qkps.tile([128, 512], F32, tag="qkps",
                                                 name=f"qk{side}_{pr}_{tcn}")
                                for kt in range(NKT):
                                    nc.tensor.matmul(
                                        ps_a[:], w_a[:, kt, :], xt[kt][:, sl],
                                        start=(kt == 0), stop=(kt == NKT - 1),
                                    )
                                q_tmp = ropetp.tile([128, 512], F32R, tag="qtmp",
                                                    name=f"qt{side}_{pr}_{tcn}")
                                nc.scalar.copy(q_tmp[:], ps_a[:])
                                return ps_a, q_tmp

                            def emit_rope(tcn, ps_a, q_tmp):
                                sl = slice(tcn * 512, (tcn + 1) * 512)
                                ps_b = qkps.tile([128, 512], F32, tag="qkpsb",
                                                 name=f"qkb{side}_{pr}_{tcn}")
                                # rotated copy: ps_b = signed 32-row-block swap of q
                                nc.tensor.matmul(
                                    ps_b[:], perm[:], q_tmp[:],
                                    start=True, stop=True,
                                )
                                t1 = ropetp.tile([128, 512], F32, tag="t1")
                                t2 = ropetp.tile([128, 512], F32, tag="t2")
                                nc.vector.tensor_tensor(t1[:], ps_a[:], cc[:, sl], OP.mult)
                                nc.vector.tensor_tensor(t2[:], ps_b[:], ss[:, sl], OP.mult)
                                nc.vector.tensor_tensor(
                                    qkt[4 * side + pr][:, sl], t1[:], t2[:], OP.add
                                )

                            prev_c = None
                            for tcn in range(4):
                                cur = emit_qmm(tcn)
                                if prev_c is not None:
                                    emit_rope(prev_c[0], *prev_c[1])
                                prev_c = (tcn, cur)
                            emit_rope(prev_c[0], *prev_c[1])

            if stop_after == 'qkv':
                with tc.tile_pool(name="dump", bufs=2) as dp:
                    for i in range(8):
                        for tcn in range(2):
                            d = dp.tile([128, 1024], F32, tag="d")
                            nc.vector.tensor_copy(d[:], qkt[i][:, tcn*1024:(tcn+1)*1024].bitcast(F32))
                            blk = 2*i + tcn
                            nc.sync.dma_start(out_d[blk*128:(blk+1)*128, :], d[:])
                continue
            # ---- phase C: attention + projection ----------------------------
            for _rc in range(reps_c):
              with (
                tc.tile_pool(name="wpp", bufs=1) as wpp,
                tc.tile_pool(name="atn", bufs=2) as atnp,
                tc.tile_pool(name="pt", bufs=4) as ptp,
                tc.tile_pool(name="nrm", bufs=2) as nrmp,
                tc.tile_pool(name="osb", bufs=2) as osbp,
                tc.tile_pool(name="stps", bufs=2, space="PSUM") as stps,
                tc.tile_pool(name="atps", bufs=2, space="PSUM") as atps,
            ):
                wp_sb = wpp.tile([128, 4, D], F32R, tag="wp")
                nc.sync.dma_start(wp_sb[:], wp_d.rearrange("(t p) f -> p t f", p=128))

                at_by_ih = {}

                def emit_proj(ih2):
                    tiles = at_by_ih[ih2]
                    for tl in range(8 if stop_after == 'full' else 0):
                        tt = 8 * ih2 + tl
                        pp = stps.tile([128, 1024], F32, tag="st")
                        for nch in range(2):
                            for mt in range(4):
                                nc.tensor.matmul(
                                    pp[:, nch * 512:(nch + 1) * 512],
                                    tiles[mt][:, tl * 128:(tl + 1) * 128],
                                    wp_sb[:, mt, nch * 512:(nch + 1) * 512],
                                    start=(mt == 0), stop=(mt == 3),
                                )
                        o_sb = osbp.tile([128, 1024], F32, tag="osb")
                        nc.vector.tensor_copy(o_sb[:], pp[:])
                        nc.sync.dma_start(out_d[tt * 128:(tt + 1) * 128, :], o_sb[:])

                for ih in range(2):
                    i0 = 1024 * ih
                    at_tiles = [
                        atnp.tile([128, 1024], F32R, tag=f"at{pr}", name=f"at{ih}_{pr}")
                        for pr in range(4)
                    ]
                    at_by_ih[ih] = at_tiles
                    n_jt = 8 * ih + 8

                    def jt_segs(jt):
                        j0 = 128 * jt
                        i_lo = max(i0, j0)
                        segs = []
                        lo = i_lo
                        while lo < i0 + 1024:
                            hi = min(i0 + 1024, (lo // 512 + 1) * 512)
                            segs.append((lo, hi))
                            lo = hi
                        return j0, i_lo, segs

                    for pr in range(4):
                        # two heads of the pair interleaved: even head uses PE
                        # rows 0:64, odd head rows 64:128 -> their K=64 score
                        # matmuls run concurrently on disjoint PE quadrants
                        heads = (2 * pr, 2 * pr + 1)
                        at_ps_h = {}
                        for h in heads:
                            at_ps_h[h] = atps.tile(
                                [65, 1024], F32, tag="atps", name=f"at{ih}_{h}"
                            )

                        def emit_st(h, jt):
                            r0 = 64 * (h % 2)
                            qt_ap = qkt[pr][r0:r0 + 64, :]
                            kt_ap = qkt[4 + pr][r0:r0 + 64, :]
                            j0, i_lo, segs = jt_segs(jt)
                            st = stps.tile([128, 1024], F32, tag="st",
                                           name=f"st{ih}_{h}_{jt}")
                            for (lo, hi) in segs:
                                nc.tensor.matmul(
                                    st[:, lo - i0:hi - i0],
                                    kt_ap[:, j0:j0 + 128],
                                    qt_ap[:, lo:hi],
                                    start=True, stop=True,
                                )
                            if j0 >= i0:
                                nc.vector.tensor_tensor(
                                    st[:, j0 - i0:j0 - i0 + 128],
                                    st[:, j0 - i0:j0 - i0 + 128],
                                    tri[:], OP.add,
                                )
                            pt = ptp.tile([128, 1024], F32R, tag="pt",
                                          name=f"pt{ih}_{h}_{jt}")
                            nc.scalar.activation(
                                pt[:, 0:i0 + 1024 - i_lo], st[:, i_lo - i0:], AF.Exp
                            )
                            return pt

                        def emit_pv(h, jt, pt):
                            j0, i_lo, segs = jt_segs(jt)
                            for (lo, hi) in segs:
                                last_jt = min(n_jt - 1, (hi - 1) // 128)
                                nc.tensor.matmul(
                                    at_ps_h[h][:, lo - i0:hi - i0],
                                    vo[jt][:, h, :],
                                    pt[:, lo - i_lo:hi - i_lo],
                                    start=(jt == 0), stop=(jt == last_jt),
                                )

                        prev = {h: None for h in heads}
                        for jt in range(n_jt):
                            pts = {}
                            for h in heads:
                                pts[h] = emit_st(h, jt)
                            for h in heads:
                                if prev[h] is not None:
                                    emit_pv(h, jt - 1, prev[h])
                                prev[h] = pts[h]
                        for h in heads:
                            emit_pv(h, n_jt - 1, prev[h])

                        for h in heads:
                            r0 = 64 * (h % 2)
                            sum_sb = nrmp.tile([1, 1024], F32, tag="sum")
                            r_sb = nrmp.tile([1, 1024], F32, tag="r")
                            rb_sb = nrmp.tile([64, 1024], F32, tag="rb")
                            nc.vector.tensor_copy(sum_sb[:], at_ps_h[h][64:65, :])
                            nc.vector.reciprocal_approx_fast(r_sb[:], sum_sb[:])
                            nc.gpsimd.partition_broadcast(rb_sb[:], r_sb[:])
                            nc.vector.tensor_tensor(
                                at_tiles[pr][r0:r0 + 64, :], at_ps_h[h][0:64, :],
                                rb_sb[:], OP.mult,
                            )
                        if ih == 1 and pr == 0:
                            emit_proj(0)
                    # proj(0) is emitted inside ih=1 (after its first head
                    # pair) so its matmuls fill PE idle time under the
                    # ACT-bound second-half attention; proj(1) at the end.
                    if ih == 1:
                        emit_proj(1)
                    if stop_after == 'attn':
                        for pr in range(4):
                            d = osbp.tile([128, 1024], F32, tag="osb")
                            nc.vector.tensor_copy(d[:], at_tiles[pr][:].bitcast(F32))
                            nc.sync.dma_start(
                                out_d[(8*ih+2*pr)*128:(8*ih+2*pr+1)*128, :], d[:])
    nc.compile()
    return nc


_NC_CACHE = None


def _get_program():
    global _NC_CACHE
    if _NC_CACHE is None:
        _NC_CACHE = _build_program()
    return _NC_CACHE


def _host_inputs(x, cos, sin, w_qkv, w_proj):
    """Build the 8 per-core input dicts."""
    x = np.asarray(x, np.float32)
    cos = np.asarray(cos, np.float32)
    sin = np.asarray(sin, np.float32)
    w_qkv = np.asarray(w_qkv, np.float32)
    w_proj = np.asarray(w_proj, np.float32)

    cct = np.tile(cos.T, (4, 1)).astype(np.float32)          # [128, T]
    sst = np.tile(sin.T, (4, 1)).astype(np.float32)
    tri = np.where(
        np.arange(128)[None, :] >= np.arange(128)[:, None], 0.0, NEG
    ).astype(np.float32)
    # signed 32-row-block swap (as matmul lhsT): out[m] = s(m) * in[src(m)]
    perm = np.zeros((128, 128), np.float32)
    for m in range(128):
        if m % 64 < 32:
            perm[m + 32, m] = -1.0
        else:
            perm[m - 32, m] = 1.0
    ident = np.eye(128, dtype=np.float32)

    x_r = [_round_fp32r(np.ascontiguousarray(x[b])) for b in range(B)]

    wq = w_qkv[:, 0:D]
    wk = w_qkv[:, D:2 * D] * np.float32(1.0 / np.sqrt(HD))
    wv = w_qkv[:, 2 * D:3 * D]

    def build_qk_aug(g):
        cols = []
        # ft 0..3: q pairs, ft 4..7: k pairs (evens then odds per head)
        for w in (wq, wk):
            for pr in range(4):
                for hl in (0, 1):
                    hw = w[:, (g * 8 + 2 * pr + hl) * 64:(g * 8 + 2 * pr + hl + 1) * 64]
                    ev, od = hw[:, 0::2], hw[:, 1::2]
                    cols.append(np.concatenate([ev, od], axis=1))
        return np.concatenate(cols, axis=1)  # [D, 1024]

    wqk_g = [_round_fp32r(build_qk_aug(g)) for g in range(2)]
    wv_g = [_round_fp32r(np.ascontiguousarray(wv[:, g * 512:(g + 1) * 512]))
            for g in range(2)]
    wp_g = [_round_fp32r(np.ascontiguousarray(w_proj[g * 512:(g + 1) * 512, :]))
            for g in range(2)]

    in_maps = []
    for c in range(N_CORES):
        b, g = c // 2, c % 2
        in_maps.append({
            "x": x_r[b], "wqk": wqk_g[g], "wv": wv_g[g], "wproj": wp_g[g],
            "cc": cct, "ss": sst, "tri": tri, "ident": _round_fp32r(ident),
            "perm": _round_fp32r(perm),
        })
    return in_maps


def kernel(x, cos, sin, w_qkv, w_proj):
    nc = _get_program()
    in_maps = _host_inputs(x, cos, sin, w_qkv, w_proj)
    res = run_bass_kernel_spmd(nc, in_maps, core_ids=list(range(N_CORES)))
    out = np.empty((B, T, D), dtype=np.float32)
    for b in range(B):
        out[b] = res.results[2 * b]["out"] + res.results[2 * b + 1]["out"]
    return out
